# revision 49
# baseline (speedup 1.0000x reference)
"""BinaryConv2D Trainium2 kernel (fp8 DoubleRow, weight-stationary).

Reference computation:
    out = conv2d(sign(x), sign(w), SAME, stride 1)   # sign(v) = +1 if v>=0 else -1
    x: (64, 56, 56, 128) f32, w: (3, 3, 128, 256) f32 -> out (64, 56, 56, 256) f32

Strategy (data-parallel over batch, 8 images per NeuronCore; per-core output
is produced channel-major and the host gather re-interleaves to NHWC):
  1. SWDGE cast-DMA x f32 -> bf16 (HBM->HBM) per image, then HW xbar
     DMA-transpose (DRAM->SBUF) [3136 px, 128 ch] -> [128 ch, 3136 px].
  2. One DVE tensor_scalar per image maps x to +-0.5 in fp8e4
     ((v >= 0) - 0.5) while scattering rows into a zero-padded 58x58 plane
     (SAME padding becomes pointer shifts).  Weights are host-binarized to
     +-1 fp8e4; the overall x2 scale is folded into the PSUM drain.
  3. Conv contracts 9 taps x 128 ci.  Taps are processed in 4 pairs via
     fp8 DoubleRow matmuls (2 MACs/cell/cycle, contraction 256) plus one
     normal fp8 matmul, weight-stationary: lhsT = w[128ci, (2tap), 128co],
     moving rhs = two tap-shifted x windows [128ci, (2, N)] streamed from
     the padded plane, accumulating PSUM [128co, N<=512px] in f32 (exact).
  4. PSUM tiles are drained with a x2 scale to bf16 (alternating ScalarE /
     VectorE so neither engine is a bottleneck) and written to HBM as
     [img, co_half, 128co, 3248 px-run]; the host strips the padding
     columns and transposes to NHWC f32 (error ~2^-9, integer output).

Built on bacc.Bacc so multi-semaphore waits are legalized into
EventSemaphore chains.
"""

import sys

if "/opt/trn_rl_repo" not in sys.path:
    sys.path.insert(0, "/opt/trn_rl_repo")

import numpy as np

import concourse.bacc as bacc
import concourse.bass as bass
import concourse.mybir as mybir
from concourse.bass import AP
from concourse.tile import TileContext
from concourse.bass_utils import run_bass_kernel_spmd

N_CORES = 8
IMGS = 8  # images per core
H = W = 56
C = 128  # input channels (= contraction dim = SBUF partitions)
O = 256  # output channels
PW = 58  # padded row width
PPI = PW * PW  # padded pixels per image (3364), rows 0/57 + cols 0/57 are pad
GUARD = 59  # zero guard on both sides of the padded plane (max |tap shift|)
XPW = GUARD + PPI + GUARD  # SBUF padded-plane width (3482)
RUN = PW * H  # contiguous output px run [row1..row56] = 3248
NT = 512  # px per PSUM tile
TILES = (RUN + NT - 1) // NT  # 7 (6x512 + 176)
F32 = mybir.dt.float32
BF16 = mybir.dt.bfloat16
FP8 = mybir.dt.float8e4

# tap order k = 3*di + dj ; shift in padded flat coords
TAP_SHIFTS = [PW * (di - 1) + (dj - 1) for di in range(3) for dj in range(3)]
# 4 DoubleRow pairs (taps 2p, 2p+1) + single tap 8
PAIR_S0 = [TAP_SHIFTS[2 * p] for p in range(4)]
PAIR_DS = [TAP_SHIFTS[2 * p + 1] - TAP_SHIFTS[2 * p] for p in range(4)]
S8 = TAP_SHIFTS[8]


CP = 112  # SBUF partitions for input chunk staging (rows*56*128 = CP*free)
CHUNKS0 = [16, 16, 16, 8]  # image-row chunks for image 0 (low first-MM latency)
CHUNKS = [56]  # image-row chunks for the rest (latency hidden by lookahead)
U16 = mybir.dt.uint16


def build_nc() -> bass.Bass:
    nc = bacc.Bacc()
    x_t = nc.dram_tensor("x", [IMGS, H * W, C], F32, kind="ExternalInput")
    # host-binarized weights [ci, 4*(2tap x 256co) + 256co] fp8e4
    wq_t = nc.dram_tensor("wq", [C, 9 * O], FP8, kind="ExternalInput")
    id_t = nc.dram_tensor("ident", [C, C], BF16, kind="ExternalInput")
    y_t = nc.dram_tensor("out", [IMGS, 2, C, RUN], BF16, kind="ExternalOutput")
    # bounce: row P = pixel pair (2P, 2P+1), col = ci, u16 = 2 fp8 pixels
    xb_ts = [nc.dram_tensor(f"xb{i}", [H * W // 2, C], U16) for i in range(IMGS)]

    with TileContext(nc) as tc:
        with (
            tc.tile_pool(name="const", bufs=1) as constp,
            tc.tile_pool(name="xld", bufs=5) as xldp,
            tc.tile_pool(name="xq", bufs=5) as xqp,
            tc.tile_pool(name="xtr", bufs=5) as xtrp,
            tc.tile_pool(name="xpad", bufs=5) as xpadp,
            tc.tile_pool(name="ostage", bufs=3) as ostagep,
            tc.tile_pool(name="psum", bufs=7, space="PSUM") as psump,
            tc.tile_pool(name="ptr", bufs=1, space="PSUM") as ptrp,
        ):
            wt = constp.tile([C, 9 * O], FP8)
            nc.sync.dma_start(out=wt[:], in_=wq_t[:])
            identb = constp.tile([C, C], BF16)
            nc.sync.dma_start(out=identb[:], in_=id_t[:])

            def w_pair_ap(p: int, h: int) -> AP:
                # [ci, (2 taps), (128 co)] slice of the pair-p block
                return wt[:, 512 * p : 512 * p + 512].rearrange(
                    "c (two co) -> c two co", two=2
                )[:, :, 128 * h : 128 * h + 128]

            def prep0() -> AP:
                """Image 0 via PE-mode transposes: the xbar transpose path
                has ~14us of startup latency; TensorE is idle at the head
                and this also warms the HAM clock early."""
                xp = xpadp.tile([C, XPW], FP8)
                nc.gpsimd.memset(xp[:, 0 : GUARD + PW + 1], 0.0)
                nc.gpsimd.memset(
                    xp[:, GUARD + 2 * PW - 1 : GUARD + 2 * PW - 1 + 55 * PW]
                    .rearrange("c (r w) -> c r w", w=PW)[:, :, 0:2],
                    0.0,
                )
                nc.gpsimd.memset(xp[:, GUARD + PPI - PW - 1 : XPW], 0.0)
                r0 = 0
                for rows in CHUNKS0:
                    npx = rows * W
                    px0 = r0 * W
                    P = 128 if npx % 128 == 0 else 64
                    nb = npx // P
                    xld = xldp.tile([P, nb * C], F32)
                    nc.scalar.dma_start(
                        out=xld[:],
                        in_=x_t[0][px0 : px0 + npx].rearrange(
                            "(b p) c -> p b c", p=P
                        ),
                    )
                    xq = xqp.tile([P, nb * C], BF16)
                    nc.vector.tensor_scalar(
                        xq[:],
                        xld[:],
                        0.0,
                        0.5,
                        op0=mybir.AluOpType.is_ge,
                        op1=mybir.AluOpType.subtract,
                    )
                    pt = ptrp.tile([C, npx], BF16)
                    for b in range(nb):
                        nc.tensor.transpose(
                            pt[:, b * P : (b + 1) * P],
                            xq[:, b * C : (b + 1) * C],
                            identb[:P, :P],
                        )
                    dst = xp[
                        :, GUARD + PW + 1 + PW * r0 : GUARD + PW + 1 + PW * (r0 + rows)
                    ].rearrange("c (r w) -> c r w", w=PW)[:, :, 0:W]
                    nc.vector.tensor_copy(
                        dst, pt[:].rearrange("c (r w) -> c r w", w=W)
                    )
                    r0 += rows
                return xp

            def prep(i: int) -> AP:
                """Input pipeline for image i: HWDGE ld f32 -> DVE binarize
                to +-0.5 fp8 with pixel pairs byte-interleaved -> HWDGE st
                -> u16 xbar transpose straight into the padded plane."""
                xp = xpadp.tile([C, XPW], FP8)
                xpu = xp[:].bitcast(U16)  # [C, XPW//2]
                # zero the guards + SAME-padding ring (disjoint from data);
                # GpSimd is otherwise idle
                nc.gpsimd.memset(xp[:, 0 : GUARD + PW + 1], 0.0)
                nc.gpsimd.memset(
                    xp[:, GUARD + 2 * PW - 1 : GUARD + 2 * PW - 1 + 55 * PW]
                    .rearrange("c (r w) -> c r w", w=PW)[:, :, 0:2],
                    0.0,
                )
                nc.gpsimd.memset(xp[:, GUARD + PPI - PW - 1 : XPW], 0.0)
                xtr = xtrp.tile([C, H * W // 2], U16)
                chunks = [28, 28] if i < 3 else CHUNKS
                bounds = [0]
                for rows in chunks:
                    bounds.append(bounds[-1] + rows)
                # phase 1: all loads first, so the sync queue never stalls
                # head-of-line on a store waiting for the DVE binarize
                xlds = []
                for ci_, rows in enumerate(chunks):
                    npx = rows * W
                    px0 = bounds[ci_] * W
                    nf = npx * C // CP  # f32 per staging partition
                    xld = xldp.tile([CP, nf], F32)
                    nc.scalar.dma_start(
                        out=xld[:],
                        in_=x_t[i][px0 : px0 + npx].rearrange(
                            "(p q) c -> p (q c)", p=CP
                        ),
                    )
                    xlds.append(xld)
                # phase 2: binarize -> store -> transpose -> scatter
                for ci_, rows in enumerate(chunks):
                    npx = rows * W
                    px0 = bounds[ci_] * W
                    r0 = bounds[ci_]
                    nf = npx * C // CP
                    xq = xqp.tile([CP, nf], FP8)
                    # (x >= 0) - 0.5 -> +-0.5 fp8; write px pairs adjacent:
                    # byte = pair*256 + ci*2 + (px%2)
                    nc.vector.tensor_scalar(
                        xq[:].rearrange("p (pair ci two) -> p pair two ci", two=2, ci=C),
                        xlds[ci_][:].rearrange(
                            "p (pair two ci) -> p pair two ci", two=2, ci=C
                        ),
                        0.0,
                        0.5,
                        op0=mybir.AluOpType.is_ge,
                        op1=mybir.AluOpType.subtract,
                    )
                    nc.gpsimd.dma_start(
                        out=xb_ts[i][px0 // 2 : (px0 + npx) // 2].rearrange(
                            "(p q) c -> p (q c)", p=CP
                        ),
                        in_=xq[:].bitcast(U16),
                    )
                    # u16 transpose -> contiguous [ci, px pairs] staging
                    nc.sync.dma_start(
                        out=xtr[:, px0 // 2 : (px0 + npx) // 2],
                        in_=xb_ts[i][px0 // 2 : (px0 + npx) // 2],
                        transpose=True,
                    )
                    # DVE scatter of pixel-pair u16s into the padded plane
                    # rows (u16 slots 59+29r .. +28, cols 1..56)
                    dstv = xpu[
                        :, 59 + 29 * r0 : 59 + 29 * (r0 + rows)
                    ].rearrange("c (r w) -> c r w", w=29)[:, :, 0:28]
                    srcv = xtr[:, px0 // 2 : (px0 + npx) // 2].rearrange(
                        "c (r w) -> c r w", w=28
                    )
                    nc.vector.tensor_copy(dstv, srcv)
                return xp

            def conv(i: int, xp: AP):
                # out px j in [0, RUN) is padded-plane px p = j + PW + 1
                # (j = 58*(r-1) + (w-1) for row r, col w) -> SBUF idx j + base
                base = GUARD + PW + 1
                for h in range(2):
                    ostage = ostagep.tile([C, RUN], BF16)
                    for t in range(TILES):
                        p0 = NT * t
                        n = min(NT, RUN - p0)
                        ps = psump.tile([C, NT], F32)
                        for p in range(4):
                            rhs = AP(
                                xp.tensor,
                                xp[:, 0:1].offset + base + p0 + PAIR_S0[p],
                                [[XPW, C], [PAIR_DS[p], 2], [1, n]],
                            )
                            nc.tensor.matmul(
                                ps[:, :n],
                                w_pair_ap(p, h),
                                rhs,
                                start=(p == 0),
                                stop=False,
                                perf_mode=mybir.MatmulPerfMode.DoubleRow,
                            )
                        a8 = base + p0 + S8
                        nc.tensor.matmul(
                            ps[:, :n],
                            wt[:, 8 * O + 128 * h : 8 * O + 128 * h + 128],
                            xp[:, a8 : a8 + n],
                            start=False,
                            stop=True,
                        )
                        # drain with the x2 binarization scale, alternating
                        # ScalarE / VectorE
                        if t % 2 == 0:
                            nc.scalar.mul(ostage[:, p0 : p0 + n], ps[:, :n], 2.0)
                        else:
                            nc.vector.tensor_scalar_mul(
                                ostage[:, p0 : p0 + n], ps[:, :n], 2.0
                            )
                    nc.scalar.dma_start(out=y_t[i][h], in_=ostage[:])

            # software-pipeline with lookahead 2: image i+1/i+2 prep is
            # emitted before image i's conv so per-engine FIFOs never
            # serialize prep behind drains, and input latency is hidden
            xps = {0: prep0(), 1: prep(1), 2: prep(2)}
            for i in range(IMGS):
                if i + 3 < IMGS:
                    xps[i + 3] = prep(i + 3)
                conv(i, xps.pop(i))

    nc.finalize()
    return nc


_NC_CACHE = None


def _get_nc():
    global _NC_CACHE
    if _NC_CACHE is None:
        _NC_CACHE = build_nc()
    return _NC_CACHE


def prep_wq(w: np.ndarray) -> np.ndarray:
    """Binarize + lay out weights on host: (3,3,128,256) f32 ->
    [128ci, 4 pair-blocks of (2 taps x 256 co) + 256 co] fp8e4 +-1."""
    import ml_dtypes

    wb = np.where(w >= 0, np.float32(1.0), np.float32(-1.0))
    # [di, dj, ci, co] -> [tap, ci, co]
    taps = wb.reshape(9, C, O)
    wq = np.empty((C, 9 * O), dtype=np.float32)
    for p in range(4):
        wq[:, 512 * p : 512 * p + 256] = taps[2 * p]
        wq[:, 512 * p + 256 : 512 * p + 512] = taps[2 * p + 1]
    wq[:, 8 * O : 9 * O] = taps[8]
    return np.ascontiguousarray(wq.astype(ml_dtypes.float8_e4m3))


def _ntff_hook():
    """NTFF capture context manager via the axon PJRT .so (the installed
    antenv lacks axon_hooks, so build the ctypes hook directly)."""
    sys.path.insert(0, "/root/.axon_site")
    from trn_agent_boot.trn_boot import _ntff_profile_via_ctypes

    return _ntff_profile_via_ctypes("/opt/axon/libaxon_pjrt.so")


def run(inputs: dict, profile_dir: str | None = None):
    """Run on all 8 NeuronCores. Returns (full_output, BassKernelResults)."""
    x = np.ascontiguousarray(np.asarray(inputs["x"], dtype=np.float32))
    w = np.ascontiguousarray(np.asarray(inputs["w"], dtype=np.float32))
    assert x.shape == (N_CORES * IMGS, H, W, C), x.shape
    assert w.shape == (3, 3, C, O), w.shape

    import ml_dtypes

    nc = _get_nc()
    wq = prep_wq(w)
    ident = np.eye(C, dtype=np.float32).astype(ml_dtypes.bfloat16)
    xr = x.reshape(N_CORES, IMGS, H * W, C)
    in_maps = [{"x": xr[i], "wq": wq, "ident": ident} for i in range(N_CORES)]
    if profile_dir is not None:
        hook = _ntff_hook()
        with hook(profile_dir, [0]):
            res = run_bass_kernel_spmd(nc, in_maps, list(range(N_CORES)))
    else:
        res = run_bass_kernel_spmd(nc, in_maps, list(range(N_CORES)))

    # device layout [img, co_half, 128co, 3248] -> NHWC f32
    out = np.empty((N_CORES * IMGS, H, W, O), dtype=np.float32)
    for c in range(N_CORES):
        yq = np.asarray(res.results[c]["out"]).astype(np.float32)
        v = yq.reshape(IMGS, 2, C, H, PW)[..., :W]  # strip pad cols
        out[c * IMGS : (c + 1) * IMGS] = v.transpose(0, 3, 4, 1, 2).reshape(
            IMGS, H, W, O
        )
    return out, res


def kernel(**inputs: np.ndarray) -> np.ndarray:
    out, _ = run(inputs)
    return out


# revision 54
# speedup vs baseline: 1.0716x; 1.0716x over previous
"""BinaryConv2D Trainium2 kernel (fp8 DoubleRow, weight-stationary).

Reference computation:
    out = conv2d(sign(x), sign(w), SAME, stride 1)   # sign(v) = +1 if v>=0 else -1
    x: (64, 56, 56, 128) f32, w: (3, 3, 128, 256) f32 -> out (64, 56, 56, 256) f32

Strategy (data-parallel over batch, 8 images per NeuronCore; per-core output
is produced channel-major and the host gather re-interleaves to NHWC):
  1. SWDGE cast-DMA x f32 -> bf16 (HBM->HBM) per image, then HW xbar
     DMA-transpose (DRAM->SBUF) [3136 px, 128 ch] -> [128 ch, 3136 px].
  2. One DVE tensor_scalar per image maps x to +-0.5 in fp8e4
     ((v >= 0) - 0.5) while scattering rows into a zero-padded 58x58 plane
     (SAME padding becomes pointer shifts).  Weights are host-binarized to
     +-1 fp8e4; the overall x2 scale is folded into the PSUM drain.
  3. Conv contracts 9 taps x 128 ci.  Taps are processed in 4 pairs via
     fp8 DoubleRow matmuls (2 MACs/cell/cycle, contraction 256) plus one
     normal fp8 matmul, weight-stationary: lhsT = w[128ci, (2tap), 128co],
     moving rhs = two tap-shifted x windows [128ci, (2, N)] streamed from
     the padded plane, accumulating PSUM [128co, N<=512px] in f32 (exact).
  4. PSUM tiles are drained with a x2 scale to bf16 (alternating ScalarE /
     VectorE so neither engine is a bottleneck) and written to HBM as
     [img, co_half, 128co, 3248 px-run]; the host strips the padding
     columns and transposes to NHWC f32 (error ~2^-9, integer output).

Built on bacc.Bacc so multi-semaphore waits are legalized into
EventSemaphore chains.
"""

import sys

if "/opt/trn_rl_repo" not in sys.path:
    sys.path.insert(0, "/opt/trn_rl_repo")

import numpy as np

import concourse.bacc as bacc
import concourse.bass as bass
import concourse.mybir as mybir
from concourse.bass import AP
from concourse.tile import TileContext
from concourse.bass_utils import run_bass_kernel_spmd

N_CORES = 8
IMGS = 8  # images per core
H = W = 56
C = 128  # input channels (= contraction dim = SBUF partitions)
O = 256  # output channels
PW = 58  # padded row width
PPI = PW * PW  # padded pixels per image (3364), rows 0/57 + cols 0/57 are pad
GUARD = 59  # zero guard on both sides of the padded plane (max |tap shift|)
XPW = GUARD + PPI + GUARD  # SBUF padded-plane width (3482)
RUN = PW * H  # contiguous output px run [row1..row56] = 3248
NT = 512  # px per PSUM tile
TILES = (RUN + NT - 1) // NT  # 7 (6x512 + 176)
F32 = mybir.dt.float32
BF16 = mybir.dt.bfloat16
FP8 = mybir.dt.float8e4

# tap order k = 3*di + dj ; shift in padded flat coords
TAP_SHIFTS = [PW * (di - 1) + (dj - 1) for di in range(3) for dj in range(3)]
# 4 DoubleRow pairs (taps 2p, 2p+1) + single tap 8
PAIR_S0 = [TAP_SHIFTS[2 * p] for p in range(4)]
PAIR_DS = [TAP_SHIFTS[2 * p + 1] - TAP_SHIFTS[2 * p] for p in range(4)]
S8 = TAP_SHIFTS[8]


CP = 112  # SBUF partitions for input chunk staging (rows*56*128 = CP*free)
CHUNKS0 = [16, 16, 16, 8]  # image-row chunks for image 0 (low first-MM latency)
CHUNKS = [56]  # image-row chunks for the rest (latency hidden by lookahead)
U16 = mybir.dt.uint16


def build_nc() -> bass.Bass:
    nc = bacc.Bacc()
    x_t = nc.dram_tensor("x", [IMGS, H * W, C], F32, kind="ExternalInput")
    # host-binarized weights [ci, 4*(2tap x 256co) + 256co] fp8e4
    wq_t = nc.dram_tensor("wq", [C, 9 * O], FP8, kind="ExternalInput")
    id_t = nc.dram_tensor("ident", [C, C], BF16, kind="ExternalInput")
    y_t = nc.dram_tensor("out", [IMGS, 2, C, RUN], BF16, kind="ExternalOutput")
    # bounce: row P = pixel pair (2P, 2P+1), col = ci, u16 = 2 fp8 pixels
    xb_ts = [nc.dram_tensor(f"xb{i}", [H * W // 2, C], U16) for i in range(IMGS)]

    with TileContext(nc) as tc:
        with (
            tc.tile_pool(name="const", bufs=1) as constp,
            tc.tile_pool(name="xld", bufs=5) as xldp,
            tc.tile_pool(name="xq", bufs=5) as xqp,
            tc.tile_pool(name="xtr", bufs=5) as xtrp,
            tc.tile_pool(name="xpad", bufs=5) as xpadp,
            tc.tile_pool(name="ostage", bufs=3) as ostagep,
            tc.tile_pool(name="psum", bufs=7, space="PSUM") as psump,
            tc.tile_pool(name="ptr", bufs=1, space="PSUM") as ptrp,
        ):
            wt = constp.tile([C, 9 * O], FP8)
            nc.sync.dma_start(out=wt[:], in_=wq_t[:])
            identb = constp.tile([C, C], BF16)
            nc.sync.dma_start(out=identb[:], in_=id_t[:])

            def w_pair_ap(p: int, h: int) -> AP:
                # [ci, (2 taps), (128 co)] slice of the pair-p block
                return wt[:, 512 * p : 512 * p + 512].rearrange(
                    "c (two co) -> c two co", two=2
                )[:, :, 128 * h : 128 * h + 128]

            def prep0() -> AP:
                """Image 0 via PE-mode transposes: the xbar transpose path
                has ~14us of startup latency; TensorE is idle at the head
                and this also warms the HAM clock early."""
                xp = xpadp.tile([C, XPW], FP8)
                nc.gpsimd.memset(xp[:, 0 : GUARD + PW + 1], 0.0)
                nc.gpsimd.memset(
                    xp[:, GUARD + 2 * PW - 1 : GUARD + 2 * PW - 1 + 55 * PW]
                    .rearrange("c (r w) -> c r w", w=PW)[:, :, 0:2],
                    0.0,
                )
                nc.gpsimd.memset(xp[:, GUARD + PPI - PW - 1 : XPW], 0.0)
                r0 = 0
                for rows in CHUNKS0:
                    npx = rows * W
                    px0 = r0 * W
                    P = 128 if npx % 128 == 0 else 64
                    nb = npx // P
                    xld = xldp.tile([P, nb * C], F32)
                    nc.sync.dma_start(
                        out=xld[:],
                        in_=x_t[0][px0 : px0 + npx].rearrange(
                            "(b p) c -> p b c", p=P
                        ),
                    )
                    xq = xqp.tile([P, nb * C], BF16)
                    nc.vector.tensor_scalar(
                        xq[:],
                        xld[:],
                        0.0,
                        0.5,
                        op0=mybir.AluOpType.is_ge,
                        op1=mybir.AluOpType.subtract,
                    )
                    pt = ptrp.tile([C, npx], BF16)
                    for b in range(nb):
                        nc.tensor.transpose(
                            pt[:, b * P : (b + 1) * P],
                            xq[:, b * C : (b + 1) * C],
                            identb[:P, :P],
                        )
                    dst = xp[
                        :, GUARD + PW + 1 + PW * r0 : GUARD + PW + 1 + PW * (r0 + rows)
                    ].rearrange("c (r w) -> c r w", w=PW)[:, :, 0:W]
                    # ScalarE copy keeps the DVE free to start image 1/2
                    # binarizes promptly
                    nc.scalar.copy(dst, pt[:].rearrange("c (r w) -> c r w", w=W))
                    r0 += rows
                return xp

            def prep(i: int) -> AP:
                """Input pipeline for image i: HWDGE ld f32 -> DVE binarize
                to +-0.5 fp8 with pixel pairs byte-interleaved -> HWDGE st
                -> u16 xbar transpose straight into the padded plane."""
                xp = xpadp.tile([C, XPW], FP8)
                xpu = xp[:].bitcast(U16)  # [C, XPW//2]
                # zero the guards + SAME-padding ring (disjoint from data);
                # GpSimd is otherwise idle
                nc.gpsimd.memset(xp[:, 0 : GUARD + PW + 1], 0.0)
                nc.gpsimd.memset(
                    xp[:, GUARD + 2 * PW - 1 : GUARD + 2 * PW - 1 + 55 * PW]
                    .rearrange("c (r w) -> c r w", w=PW)[:, :, 0:2],
                    0.0,
                )
                nc.gpsimd.memset(xp[:, GUARD + PPI - PW - 1 : XPW], 0.0)
                xtr = xtrp.tile([C, H * W // 2], U16)
                chunks = CHUNKS
                bounds = [0]
                for rows in chunks:
                    bounds.append(bounds[-1] + rows)
                # phase 1: all loads first, so the sync queue never stalls
                # head-of-line on a store waiting for the DVE binarize
                xlds = []
                for ci_, rows in enumerate(chunks):
                    npx = rows * W
                    px0 = bounds[ci_] * W
                    nf = npx * C // CP  # f32 per staging partition
                    xld = xldp.tile([CP, nf], F32)
                    nc.sync.dma_start(
                        out=xld[:],
                        in_=x_t[i][px0 : px0 + npx].rearrange(
                            "(p q) c -> p (q c)", p=CP
                        ),
                    )
                    xlds.append(xld)
                # phase 2: binarize -> store -> transpose -> scatter
                for ci_, rows in enumerate(chunks):
                    npx = rows * W
                    px0 = bounds[ci_] * W
                    r0 = bounds[ci_]
                    nf = npx * C // CP
                    xq = xqp.tile([CP, nf], FP8)
                    # (x >= 0) - 0.5 -> +-0.5 fp8; write px pairs adjacent:
                    # byte = pair*256 + ci*2 + (px%2)
                    nc.vector.tensor_scalar(
                        xq[:].rearrange("p (pair ci two) -> p pair two ci", two=2, ci=C),
                        xlds[ci_][:].rearrange(
                            "p (pair two ci) -> p pair two ci", two=2, ci=C
                        ),
                        0.0,
                        0.5,
                        op0=mybir.AluOpType.is_ge,
                        op1=mybir.AluOpType.subtract,
                    )
                    nc.sync.dma_start(
                        out=xb_ts[i][px0 // 2 : (px0 + npx) // 2].rearrange(
                            "(p q) c -> p (q c)", p=CP
                        ),
                        in_=xq[:].bitcast(U16),
                    )
                    # u16 transpose -> contiguous [ci, px pairs] staging
                    nc.sync.dma_start(
                        out=xtr[:, px0 // 2 : (px0 + npx) // 2],
                        in_=xb_ts[i][px0 // 2 : (px0 + npx) // 2],
                        transpose=True,
                    )
                    # DVE scatter of pixel-pair u16s into the padded plane
                    # rows (u16 slots 59+29r .. +28, cols 1..56)
                    dstv = xpu[
                        :, 59 + 29 * r0 : 59 + 29 * (r0 + rows)
                    ].rearrange("c (r w) -> c r w", w=29)[:, :, 0:28]
                    srcv = xtr[:, px0 // 2 : (px0 + npx) // 2].rearrange(
                        "c (r w) -> c r w", w=28
                    )
                    nc.vector.tensor_copy(dstv, srcv)
                return xp

            def conv(i: int, xp: AP):
                # out px j in [0, RUN) is padded-plane px p = j + PW + 1
                # (j = 58*(r-1) + (w-1) for row r, col w) -> SBUF idx j + base
                base = GUARD + PW + 1
                for h in range(2):
                    ostage = ostagep.tile([C, RUN], BF16)
                    for t in range(TILES):
                        p0 = NT * t
                        n = min(NT, RUN - p0)
                        ps = psump.tile([C, NT], F32)
                        for p in range(4):
                            rhs = AP(
                                xp.tensor,
                                xp[:, 0:1].offset + base + p0 + PAIR_S0[p],
                                [[XPW, C], [PAIR_DS[p], 2], [1, n]],
                            )
                            nc.tensor.matmul(
                                ps[:, :n],
                                w_pair_ap(p, h),
                                rhs,
                                start=(p == 0),
                                stop=False,
                                perf_mode=mybir.MatmulPerfMode.DoubleRow,
                            )
                        a8 = base + p0 + S8
                        nc.tensor.matmul(
                            ps[:, :n],
                            wt[:, 8 * O + 128 * h : 8 * O + 128 * h + 128],
                            xp[:, a8 : a8 + n],
                            start=False,
                            stop=True,
                        )
                        # drain with the x2 binarization scale, alternating
                        # ScalarE / VectorE
                        if t % 2 == 0:
                            nc.scalar.mul(ostage[:, p0 : p0 + n], ps[:, :n], 2.0)
                        else:
                            nc.vector.tensor_scalar_mul(
                                ostage[:, p0 : p0 + n], ps[:, :n], 2.0
                            )
                    nc.scalar.dma_start(out=y_t[i][h], in_=ostage[:])

            # software-pipeline with lookahead 2: image i+1/i+2 prep is
            # emitted before image i's conv so per-engine FIFOs never
            # serialize prep behind drains, and input latency is hidden
            xps = {0: prep0(), 1: prep(1), 2: prep(2)}
            for i in range(IMGS):
                if i + 3 < IMGS:
                    xps[i + 3] = prep(i + 3)
                conv(i, xps.pop(i))

    nc.finalize()
    return nc


_NC_CACHE = None


def _get_nc():
    global _NC_CACHE
    if _NC_CACHE is None:
        _NC_CACHE = build_nc()
    return _NC_CACHE


def prep_wq(w: np.ndarray) -> np.ndarray:
    """Binarize + lay out weights on host: (3,3,128,256) f32 ->
    [128ci, 4 pair-blocks of (2 taps x 256 co) + 256 co] fp8e4 +-1."""
    import ml_dtypes

    wb = np.where(w >= 0, np.float32(1.0), np.float32(-1.0))
    # [di, dj, ci, co] -> [tap, ci, co]
    taps = wb.reshape(9, C, O)
    wq = np.empty((C, 9 * O), dtype=np.float32)
    for p in range(4):
        wq[:, 512 * p : 512 * p + 256] = taps[2 * p]
        wq[:, 512 * p + 256 : 512 * p + 512] = taps[2 * p + 1]
    wq[:, 8 * O : 9 * O] = taps[8]
    return np.ascontiguousarray(wq.astype(ml_dtypes.float8_e4m3))


def _ntff_hook():
    """NTFF capture context manager via the axon PJRT .so (the installed
    antenv lacks axon_hooks, so build the ctypes hook directly)."""
    sys.path.insert(0, "/root/.axon_site")
    from trn_agent_boot.trn_boot import _ntff_profile_via_ctypes

    return _ntff_profile_via_ctypes("/opt/axon/libaxon_pjrt.so")


def run(inputs: dict, profile_dir: str | None = None):
    """Run on all 8 NeuronCores. Returns (full_output, BassKernelResults)."""
    x = np.ascontiguousarray(np.asarray(inputs["x"], dtype=np.float32))
    w = np.ascontiguousarray(np.asarray(inputs["w"], dtype=np.float32))
    assert x.shape == (N_CORES * IMGS, H, W, C), x.shape
    assert w.shape == (3, 3, C, O), w.shape

    import ml_dtypes

    nc = _get_nc()
    wq = prep_wq(w)
    ident = np.eye(C, dtype=np.float32).astype(ml_dtypes.bfloat16)
    xr = x.reshape(N_CORES, IMGS, H * W, C)
    in_maps = [{"x": xr[i], "wq": wq, "ident": ident} for i in range(N_CORES)]
    if profile_dir is not None:
        hook = _ntff_hook()
        with hook(profile_dir, [0]):
            res = run_bass_kernel_spmd(nc, in_maps, list(range(N_CORES)))
    else:
        res = run_bass_kernel_spmd(nc, in_maps, list(range(N_CORES)))

    # device layout [img, co_half, 128co, 3248] -> NHWC f32
    out = np.empty((N_CORES * IMGS, H, W, O), dtype=np.float32)
    for c in range(N_CORES):
        yq = np.asarray(res.results[c]["out"]).astype(np.float32)
        v = yq.reshape(IMGS, 2, C, H, PW)[..., :W]  # strip pad cols
        out[c * IMGS : (c + 1) * IMGS] = v.transpose(0, 3, 4, 1, 2).reshape(
            IMGS, H, W, O
        )
    return out, res


def kernel(**inputs: np.ndarray) -> np.ndarray:
    out, _ = run(inputs)
    return out
